# revision 1
# baseline (speedup 1.0000x reference)
"""DLRM forward (embedding gather + tiny MLPs) as a Bass/Tile kernel on 8 trn2 cores.

Sharding: data-parallel over the batch. Each of the 8 cores gets B/8 = 2048 rows
of dense_x / sparse_x plus a full replica of the (read-only) embedding tables,
computes its batch shard end-to-end on device, and returns [1, 2048] sigmoid
outputs. The host only slices inputs and concatenates outputs.

Per-core pipeline:
  - combined index = sparse_idx + f*CARD (iota + DVE add), tables viewed flat
    [26*100000, 64] so one indirect DMA per 128-row batch tile gathers all 26
    embedding rows per sample: [128, 26] idxs -> [128, 1664] f32.
  - PE transposes 128x128 feature chunks (features -> partitions), DVE/ACT
    copy-casts PSUM -> SBUF bf16, PE matmuls against bf16 tw1 chunks accumulate
    the top-MLP hidden layer [16, 512] per 512-sample group.
  - bottom MLP runs transposed ([13,512] -> [8,512] -> [64,512]) and feeds the
    last accumulation chunk. ACT applies biases/relu/sigmoid.
"""

import numpy as np

import concourse.bass as bass
import concourse.mybir as mybir
import concourse.tile as tile
from concourse import bacc
from concourse.masks import make_identity

P = 128

# Problem constants (hardcoded per harness contract).
N_CORES = 8
B = 16384
F = 26
D = 64
DENSE = 13
CARD = 100000
H_BOT = 8
H_TOP = 16

f32 = mybir.dt.float32
i32 = mybir.dt.int32
bf16 = mybir.dt.bfloat16
fp16 = mybir.dt.float16


def build_kernel(
    b_loc=B // N_CORES,
    card=CARD,
    n_f=F,
    d=D,
    n_dense=DENSE,
    h_bot=H_BOT,
    h_top=H_TOP,
    compute_dt=fp16,
    table_dt=fp16,
):
    v = n_f * card
    k_emb = n_f * d
    assert k_emb % P == 0
    kc_n = k_emb // P  # feature chunks of 128
    group = min(512, b_loc)  # batch columns per matmul group
    tpg = group // P  # 128-row tiles per group
    n_g = b_loc // group
    assert b_loc % group == 0 and group % P == 0

    # Bacc (not raw Bass): its compile() pipeline legalizes semaphore waits
    # (TRN2 allows one wait per instruction) via generate_event_semaphores.
    nc = bacc.Bacc("TRN2", target_bir_lowering=False)
    dense_d = nc.dram_tensor("dense_x", [b_loc, n_dense], f32, kind="ExternalInput")
    sparse_d = nc.dram_tensor("sparse_x", [b_loc, n_f], i32, kind="ExternalInput")
    tables_d = nc.dram_tensor("tables", [v, d], table_dt, kind="ExternalInput")
    w1_d = nc.dram_tensor("w1", [n_dense, h_bot], f32, kind="ExternalInput")
    b1_d = nc.dram_tensor("b1", [h_bot], f32, kind="ExternalInput")
    w2_d = nc.dram_tensor("w2", [h_bot, d], f32, kind="ExternalInput")
    b2_d = nc.dram_tensor("b2", [d], f32, kind="ExternalInput")
    tw1_d = nc.dram_tensor("tw1", [k_emb + d, h_top], f32, kind="ExternalInput")
    tb1_d = nc.dram_tensor("tb1", [h_top], f32, kind="ExternalInput")
    tw2_d = nc.dram_tensor("tw2", [h_top, 1], f32, kind="ExternalInput")
    tb2_d = nc.dram_tensor("tb2", [1], f32, kind="ExternalInput")
    y_d = nc.dram_tensor("y", [1, b_loc], f32, kind="ExternalOutput")

    n_t = b_loc // P

    with tile.TileContext(nc) as tc:
        with (
            tc.tile_pool(name="const", bufs=1) as cpool,
            tc.tile_pool(name="emb", bufs=6) as embp,
            tc.tile_pool(name="embT", bufs=4) as embtp,
            tc.tile_pool(name="dx", bufs=3) as dxp,
            tc.tile_pool(name="small", bufs=2) as smallp,
            tc.tile_pool(name="ptr", bufs=3, space="PSUM") as ptrp,
            tc.tile_pool(name="po1", bufs=2, space="PSUM") as po1p,
            tc.tile_pool(name="psmall", bufs=2, space="PSUM") as psmallp,
        ):
            # ---- constants / weights ----
            ident = cpool.tile([P, P], f32)
            make_identity(nc, ident[:])
            if table_dt == f32:
                ident_t = ident
            else:
                ident_t = cpool.tile([P, P], table_dt)
                make_identity(nc, ident_t[:])

            # per-sample table base offsets: fofs[p, t, f] = f * card
            # (iota pattern steps are int16-limited, so generate f then scale;
            # keep the whole chain on gpsimd — walrus allows only one sync
            # wait on TensorTensor-class instructions, and a single-engine
            # chain needs just the one DMA wait)
            fidx = cpool.tile([P, n_t * n_f], i32)
            nc.gpsimd.iota(
                fidx[:], pattern=[[0, n_t], [1, n_f]], base=0, channel_multiplier=0
            )
            fofs = cpool.tile([P, n_t * n_f], i32)
            nc.gpsimd.tensor_scalar_mul(fofs[:], fidx[:], card)
            idx_raw = cpool.tile([P, n_t * n_f], i32)
            nc.sync.dma_start(
                out=idx_raw[:].rearrange("p (t f) -> p t f", t=n_t),
                in_=sparse_d[:, :].rearrange("(t p) f -> p t f", p=P),
            )
            # TensorTensor-class instructions have a single ISA wait slot, so
            # stage through a same-engine copy: the copy absorbs the DMA wait
            # into Pool's vector clock, the add then only self-waits on Pool.
            comb = cpool.tile([P, n_t * n_f], i32)
            nc.gpsimd.tensor_copy(out=comb[:], in_=idx_raw[:])
            nc.gpsimd.tensor_tensor(
                out=comb[:], in0=comb[:], in1=fofs[:], op=mybir.AluOpType.add
            )

            tw1_f = cpool.tile([P, kc_n * h_top], f32)
            nc.sync.dma_start(
                out=tw1_f[:].rearrange("p (c m) -> p c m", c=kc_n),
                in_=tw1_d[0:k_emb, :].rearrange("(c p) m -> p c m", p=P),
            )
            tw1_c = cpool.tile([P, kc_n * h_top], compute_dt)
            nc.vector.tensor_copy(out=tw1_c[:], in_=tw1_f[:])

            tw1d_f = cpool.tile([d, h_top], f32)
            nc.sync.dma_start(out=tw1d_f[:], in_=tw1_d[k_emb : k_emb + d, :])
            tw1d_c = cpool.tile([d, h_top], compute_dt)
            nc.vector.tensor_copy(out=tw1d_c[:], in_=tw1d_f[:])

            tw2_f = cpool.tile([h_top, 1], f32)
            nc.sync.dma_start(out=tw2_f[:], in_=tw2_d[:, :])
            tw2_c = cpool.tile([h_top, 1], compute_dt)
            nc.vector.tensor_copy(out=tw2_c[:], in_=tw2_f[:])

            w1_sb = cpool.tile([n_dense, h_bot], f32)
            nc.sync.dma_start(out=w1_sb[:], in_=w1_d[:, :])
            w2_sb = cpool.tile([h_bot, d], f32)
            nc.sync.dma_start(out=w2_sb[:], in_=w2_d[:, :])
            b1_sb = cpool.tile([h_bot, 1], f32)
            nc.sync.dma_start(out=b1_sb[:], in_=b1_d[:, None])
            b2_sb = cpool.tile([d, 1], f32)
            nc.sync.dma_start(out=b2_sb[:], in_=b2_d[:, None])
            tb1_sb = cpool.tile([h_top, 1], f32)
            nc.sync.dma_start(out=tb1_sb[:], in_=tb1_d[:, None])
            tb2_sb = cpool.tile([1, 1], f32)
            nc.sync.dma_start(out=tb2_sb[:], in_=tb2_d[:, None])

            y_row = cpool.tile([1, b_loc], f32)

            for g in range(n_g):
                # ---- embedding gathers: one indirect DMA per 128-sample tile ----
                emb_tiles = []
                for j in range(tpg):
                    t = g * tpg + j
                    et = embp.tile([P, k_emb], table_dt, tag="emb")
                    nc.gpsimd.indirect_dma_start(
                        out=et[:],
                        out_offset=None,
                        in_=tables_d[:, :],
                        in_offset=bass.IndirectOffsetOnAxis(
                            ap=comb[:, t * n_f : (t + 1) * n_f], axis=0
                        ),
                    )
                    emb_tiles.append(et)

                # ---- bottom MLP (transposed layout) ----
                pdx = psmallp.tile([n_dense, group], f32, tag="psmall")
                for j in range(tpg):
                    t = g * tpg + j
                    dx_t = dxp.tile([P, n_dense], f32, tag="dx")
                    nc.sync.dma_start(out=dx_t[:], in_=dense_d[bass.ts(t, P), :])
                    nc.tensor.transpose(
                        out=pdx[:, bass.ts(j, P)], in_=dx_t[:], identity=ident[:]
                    )
                dxt = smallp.tile([n_dense, group], f32, tag="dxt")
                nc.vector.tensor_copy(out=dxt[:], in_=pdx[:])
                ph = psmallp.tile([h_bot, group], f32, tag="psmall")
                nc.tensor.matmul(out=ph[:], lhsT=w1_sb[:], rhs=dxt[:], start=True, stop=True)
                h_s = smallp.tile([h_bot, group], f32, tag="h")
                nc.scalar.activation(
                    out=h_s[:],
                    in_=ph[:],
                    func=mybir.ActivationFunctionType.Relu,
                    bias=b1_sb[:],
                )
                pd = psmallp.tile([d, group], f32, tag="psmall")
                nc.tensor.matmul(out=pd[:], lhsT=w2_sb[:], rhs=h_s[:], start=True, stop=True)
                dt_sb = smallp.tile([d, group], compute_dt, tag="dt")
                nc.scalar.activation(
                    out=dt_sb[:],
                    in_=pd[:],
                    func=mybir.ActivationFunctionType.Identity,
                    bias=b2_sb[:],
                )

                # ---- top MLP layer 1: transpose feature chunks, accumulate ----
                po1 = po1p.tile([h_top, group], f32, tag="po1")
                for kc in range(kc_n):
                    ptr = ptrp.tile([P, group], table_dt, tag="ptr")
                    for j in range(tpg):
                        nc.tensor.transpose(
                            out=ptr[:, bass.ts(j, P)],
                            in_=emb_tiles[j][:, bass.ts(kc, P)],
                            identity=ident_t[:],
                        )
                    embt = embtp.tile([P, group], compute_dt, tag="embT")
                    if kc % 2 == 0:
                        nc.vector.tensor_copy(out=embt[:], in_=ptr[:])
                    else:
                        nc.scalar.activation(
                            out=embt[:],
                            in_=ptr[:],
                            func=mybir.ActivationFunctionType.Copy,
                        )
                    nc.tensor.matmul(
                        out=po1[:],
                        lhsT=tw1_c[:, bass.ts(kc, h_top)],
                        rhs=embt[:],
                        start=(kc == 0),
                        stop=False,
                    )
                nc.tensor.matmul(
                    out=po1[:], lhsT=tw1d_c[:], rhs=dt_sb[:], start=False, stop=True
                )

                o1 = smallp.tile([h_top, group], compute_dt, tag="o1")
                nc.scalar.activation(
                    out=o1[:],
                    in_=po1[:],
                    func=mybir.ActivationFunctionType.Relu,
                    bias=tb1_sb[:],
                )
                plg = psmallp.tile([1, group], f32, tag="psmall")
                nc.tensor.matmul(out=plg[:], lhsT=tw2_c[:], rhs=o1[:], start=True, stop=True)
                nc.scalar.activation(
                    out=y_row[:, bass.ts(g, group)],
                    in_=plg[:],
                    func=mybir.ActivationFunctionType.Sigmoid,
                    bias=tb2_sb[:],
                )

            nc.sync.dma_start(out=y_d[:, :], in_=y_row[:])

    nc.compile()
    return nc


_NC_CACHE = {}


def _get_nc():
    if "nc" not in _NC_CACHE:
        _NC_CACHE["nc"] = build_kernel()
    return _NC_CACHE["nc"]


TABLE_NP_DT = np.float16


def make_in_maps(dense_x, sparse_x, tables, w1, b1, w2, b2, tw1, tb1, tw2, tb2):
    tables_flat = np.ascontiguousarray(
        np.asarray(tables).reshape(F * CARD, D).astype(TABLE_NP_DT)
    )
    sparse_i32 = np.ascontiguousarray(np.asarray(sparse_x, dtype=np.int32))
    dense_f = np.ascontiguousarray(np.asarray(dense_x, dtype=np.float32))
    shared = {
        "tables": tables_flat,
        "w1": np.ascontiguousarray(np.asarray(w1, np.float32)),
        "b1": np.ascontiguousarray(np.asarray(b1, np.float32)),
        "w2": np.ascontiguousarray(np.asarray(w2, np.float32)),
        "b2": np.ascontiguousarray(np.asarray(b2, np.float32)),
        "tw1": np.ascontiguousarray(np.asarray(tw1, np.float32)),
        "tb1": np.ascontiguousarray(np.asarray(tb1, np.float32)),
        "tw2": np.ascontiguousarray(np.asarray(tw2, np.float32)),
        "tb2": np.ascontiguousarray(np.asarray(tb2, np.float32)),
    }
    b_loc = B // N_CORES
    in_maps = []
    for c in range(N_CORES):
        m = dict(shared)
        m["dense_x"] = dense_f[c * b_loc : (c + 1) * b_loc]
        m["sparse_x"] = sparse_i32[c * b_loc : (c + 1) * b_loc]
        in_maps.append(m)
    return in_maps


def kernel(**inputs):
    from concourse.bass_utils import run_bass_kernel_spmd

    nc = _get_nc()
    in_maps = make_in_maps(**inputs)
    res = run_bass_kernel_spmd(nc, in_maps, core_ids=list(range(N_CORES)))
    out = np.concatenate([r["y"].reshape(-1) for r in res.results])
    return out.reshape(B, 1).astype(np.float32)



# revision 3
# speedup vs baseline: 2.4350x; 2.4350x over previous
"""DLRM forward (embedding gather + tiny MLPs) as a Bass/Tile kernel on 8 trn2 cores.

Sharding: data-parallel over the batch; each of the 8 cores handles B/8 = 2048
samples end-to-end against a full replica of the (read-only) tables.

Key transformation (host-side, exact): the top-MLP first layer is linear in the
embedding concat, so fold tw1 into the tables once per call:
    PT[f] = tables[f] @ tw1[f*64:(f+1)*64, :]        # [CARD, 16] per table
    hidden[b] = sum_f PT[f][idx[b,f]] + relu(x@w1+b1) @ (w2@tw1_d) + (b2@tw1_d + tb1)
The constant vector (b2@tw1_d + tb1) is folded into PT table 0. The device
then gathers 64B fp32 rows (26 per sample) and reduces them on DVE — no
[B, 26, 64] materialization, no PE transposes of embeddings, and the gather
descriptor stream (53248 64B rows/core) runs at the DMA floor.

Per-core pipeline:
  - 16 indirect DMAs (one per 128-sample tile) gather 26x16 f32 into one big
    SBUF tile; DVE reduces each [128, 26, 16] block over f (axis-X reduce on a
    strided view) into gathersum [128, 16].
  - bottom MLP runs feature-major on PE ([13,2048] loaded pre-transposed from
    host): w1 matmul -> relu -> (w2@tw1_d) matmul -> PE-transpose back to
    sample-major [128, 16] chunks in PSUM.
  - DVE adds gathersum + dense chunk, applies relu*tw2 in one
    scalar_tensor_tensor, reduces over the 16 hidden units; ACT applies
    sigmoid(+tb2); one final PE transpose lays y out [16, 128] for a single
    contiguous output DMA.
"""

import numpy as np

import concourse.bass as bass
import concourse.mybir as mybir
import concourse.tile as tile
from concourse import bacc

P = 128

# Problem constants (hardcoded per harness contract).
N_CORES = 8
B = 16384
F = 26
D = 64
DENSE = 13
CARD = 100000
H_BOT = 8
H_TOP = 16

f32 = mybir.dt.float32
i32 = mybir.dt.int32

GATHER_TILES_PER_INST = 1  # tiles of 128 samples per indirect DMA


def build_kernel(
    b_loc=B // N_CORES,
    card=CARD,
    n_f=F,
    n_dense=DENSE,
    h_bot=H_BOT,
    h_top=H_TOP,
):
    v = n_f * card
    n_t = b_loc // P  # 16 tiles of 128 samples
    group = min(512, b_loc)  # batch columns per matmul group
    tpg = group // P  # tiles per group (4)
    n_g = b_loc // group  # groups (4)
    row = h_top  # gathered row length (16 f32)
    trow = n_f * row  # per-tile gather width (416)

    nc = bacc.Bacc("TRN2", target_bir_lowering=False)
    ptab_d = nc.dram_tensor("ptab", [v, row], f32, kind="ExternalInput")
    comb_d = nc.dram_tensor("comb", [P, n_t * n_f], i32, kind="ExternalInput")
    dxt_d = nc.dram_tensor("dxt", [n_dense, b_loc], f32, kind="ExternalInput")
    w1_d = nc.dram_tensor("w1", [n_dense, h_bot], f32, kind="ExternalInput")
    b1_d = nc.dram_tensor("b1", [h_bot], f32, kind="ExternalInput")
    w21_d = nc.dram_tensor("w21", [h_bot, h_top], f32, kind="ExternalInput")
    # bcast[:, :64] = tw2 tiled 4x across partitions; bcast[:, 64] = tb2
    bc_d = nc.dram_tensor("bcast", [P, tpg * h_top + 1], f32, kind="ExternalInput")
    id_d = nc.dram_tensor("ident", [P, P], f32, kind="ExternalInput")
    y_d = nc.dram_tensor("y", [n_t, P], f32, kind="ExternalOutput")

    with tile.TileContext(nc) as tc:
        with (
            tc.tile_pool(name="const", bufs=1) as cpool,
            tc.tile_pool(name="small", bufs=2) as smallp,
            tc.tile_pool(name="pmm", bufs=2, space="PSUM") as pmmp,
            tc.tile_pool(name="pfix", bufs=1, space="PSUM") as pfixp,
        ):
            # ---- index upload, then gathers ASAP on gpsimd ----
            comb = cpool.tile([P, n_t * n_f], i32)
            nc.sync.dma_start(out=comb[:], in_=comb_d[:, :])

            big_et = cpool.tile([P, n_t * trow], f32)
            gstep = GATHER_TILES_PER_INST
            for t0 in range(0, n_t, gstep):
                t1 = min(t0 + gstep, n_t)
                nc.gpsimd.indirect_dma_start(
                    out=big_et[:, t0 * trow : t1 * trow],
                    out_offset=None,
                    in_=ptab_d[:, :],
                    in_offset=bass.IndirectOffsetOnAxis(
                        ap=comb[:, t0 * n_f : t1 * n_f], axis=0
                    ),
                )

            # ---- constants (no device-side setup compute) ----
            ident = cpool.tile([P, P], f32)
            nc.sync.dma_start(out=ident[:], in_=id_d[:, :])
            dxt = cpool.tile([n_dense, b_loc], f32)
            nc.sync.dma_start(out=dxt[:], in_=dxt_d[:, :])
            w1_sb = cpool.tile([n_dense, h_bot], f32)
            nc.sync.dma_start(out=w1_sb[:], in_=w1_d[:, :])
            b1_sb = cpool.tile([h_bot, 1], f32)
            nc.sync.dma_start(out=b1_sb[:], in_=b1_d[:, None])
            w21_sb = cpool.tile([h_bot, h_top], f32)
            nc.sync.dma_start(out=w21_sb[:], in_=w21_d[:, :])
            bc = cpool.tile([P, tpg * h_top + 1], f32)
            nc.sync.dma_start(out=bc[:], in_=bc_d[:, :])

            gs = cpool.tile([P, n_t * h_top], f32)  # gathersum [128, 256]
            hs = cpool.tile([P, n_t * h_top], f32)  # hidden pre-relu
            mm = cpool.tile([P, n_t * h_top], f32)  # relu(h) * tw2
            lg = cpool.tile([P, n_t], f32)  # logits [128, 16]
            ylog = cpool.tile([P, n_t], f32)
            pdhT = pfixp.tile([P, n_t * h_top], f32)  # dense hidden, sample-major

            # ---- bottom MLP, feature-major ----
            for g in range(n_g):
                ph = pmmp.tile([h_bot, group], f32, tag="pmm")
                nc.tensor.matmul(
                    out=ph[:],
                    lhsT=w1_sb[:],
                    rhs=dxt[:, bass.ts(g, group)],
                    start=True,
                    stop=True,
                )
                h1 = smallp.tile([h_bot, group], f32, tag="h1")
                nc.scalar.activation(
                    out=h1[:],
                    in_=ph[:],
                    func=mybir.ActivationFunctionType.Relu,
                    bias=b1_sb[:],
                )
                pd = pmmp.tile([h_top, group], f32, tag="pmm")
                nc.tensor.matmul(
                    out=pd[:], lhsT=w21_sb[:], rhs=h1[:], start=True, stop=True
                )
                dh = smallp.tile([h_top, group], f32, tag="dh")
                nc.scalar.activation(
                    out=dh[:],
                    in_=pd[:],
                    func=mybir.ActivationFunctionType.Copy,
                )
                for j in range(tpg):
                    nc.tensor.transpose(
                        out=pdhT[:, bass.ts(g * tpg + j, h_top)],
                        in_=dh[:, bass.ts(j, P)],
                        identity=ident[0:h_top, 0:h_top],
                    )

            # ---- gather reduction + head, per 128-sample tile ----
            for t in range(n_t):
                # sum over the 26 tables: [128, (f j)] -> [128, j] (f innermost)
                nc.vector.tensor_reduce(
                    out=gs[:, bass.ts(t, h_top)],
                    in_=big_et[:, bass.ts(t, trow)].rearrange(
                        "p (f j) -> p j f", f=n_f
                    ),
                    axis=mybir.AxisListType.X,
                    op=mybir.AluOpType.add,
                )
            for g in range(n_g):
                gcols = bass.ts(g, tpg * h_top)
                nc.vector.tensor_tensor(
                    out=hs[:, gcols],
                    in0=gs[:, gcols],
                    in1=pdhT[:, gcols],
                    op=mybir.AluOpType.add,
                )
                # relu then scale by tw2 (broadcast across partitions)
                nc.vector.scalar_tensor_tensor(
                    out=mm[:, gcols],
                    in0=hs[:, gcols],
                    scalar=0.0,
                    in1=bc[:, 0 : tpg * h_top],
                    op0=mybir.AluOpType.max,
                    op1=mybir.AluOpType.mult,
                )
                nc.vector.tensor_reduce(
                    out=lg[:, bass.ts(g, tpg)],
                    in_=mm[:, gcols].rearrange("p (t j) -> p t j", t=tpg),
                    axis=mybir.AxisListType.X,
                    op=mybir.AluOpType.add,
                )

            nc.scalar.activation(
                out=ylog[:],
                in_=lg[:],
                func=mybir.ActivationFunctionType.Sigmoid,
                bias=bc[:, tpg * h_top : tpg * h_top + 1],
            )
            pyT = pfixp.tile([n_t, P], f32)
            nc.tensor.transpose(out=pyT[:], in_=ylog[:], identity=ident[:])
            yT = cpool.tile([n_t, P], f32)
            nc.vector.tensor_copy(out=yT[:], in_=pyT[:])
            nc.sync.dma_start(out=y_d[:, :], in_=yT[:])

    nc.compile()
    return nc


_NC_CACHE = {}


def _get_nc():
    if "nc" not in _NC_CACHE:
        _NC_CACHE["nc"] = build_kernel()
    return _NC_CACHE["nc"]


def make_in_maps(dense_x, sparse_x, tables, w1, b1, w2, b2, tw1, tb1, tw2, tb2):
    tables = np.asarray(tables, dtype=np.float32)
    tw1 = np.asarray(tw1, dtype=np.float32)
    tw2 = np.asarray(tw2, dtype=np.float32)
    w2 = np.asarray(w2, dtype=np.float32)
    b2 = np.asarray(b2, dtype=np.float32)
    tb1 = np.asarray(tb1, dtype=np.float32)
    tb2 = np.asarray(tb2, dtype=np.float32)

    # Fold tw1 into the tables: PT[f] = tables[f] @ tw1_f  -> [F, CARD, 16]
    tw1_e = tw1[: F * D].reshape(F, D, H_TOP)
    pt = np.einsum("fcd,fdh->fch", tables, tw1_e, optimize=True).astype(np.float32)
    # Fold the constant hidden-layer offset into table 0.
    c = (b2 @ tw1[F * D :]) + tb1  # [16]
    pt[0] += c
    ptab = np.ascontiguousarray(pt.reshape(F * CARD, H_TOP))

    w21 = np.ascontiguousarray(w2 @ tw1[F * D :])  # [8, 16]

    comb_full = np.asarray(sparse_x, dtype=np.int64) + (
        np.arange(F, dtype=np.int64) * CARD
    )
    comb_full = comb_full.astype(np.int32)  # [B, 26] flat PT row ids

    dense_f = np.asarray(dense_x, dtype=np.float32)

    tpg = 4
    bc = np.empty((P, tpg * H_TOP + 1), dtype=np.float32)
    bc[:, : tpg * H_TOP] = np.tile(tw2.reshape(-1), tpg)
    bc[:, tpg * H_TOP] = tb2[0]

    ident = np.eye(P, dtype=np.float32)

    shared = {
        "ptab": ptab,
        "w1": np.ascontiguousarray(np.asarray(w1, np.float32)),
        "b1": np.ascontiguousarray(np.asarray(b1, np.float32)),
        "w21": w21,
        "bcast": bc,
        "ident": ident,
    }
    b_loc = B // N_CORES
    n_t = b_loc // P
    in_maps = []
    for c_ in range(N_CORES):
        lo, hi = c_ * b_loc, (c_ + 1) * b_loc
        comb_c = comb_full[lo:hi].reshape(n_t, P, F).transpose(1, 0, 2)
        m = dict(shared)
        m["comb"] = np.ascontiguousarray(comb_c.reshape(P, n_t * F))
        m["dxt"] = np.ascontiguousarray(dense_f[lo:hi].T)
        in_maps.append(m)
    return in_maps


def kernel(**inputs):
    from concourse.bass_utils import run_bass_kernel_spmd

    nc = _get_nc()
    in_maps = make_in_maps(**inputs)
    res = run_bass_kernel_spmd(nc, in_maps, core_ids=list(range(N_CORES)))
    out = np.concatenate([r["y"].reshape(-1) for r in res.results])
    return out.reshape(B, 1).astype(np.float32)


# revision 6
# speedup vs baseline: 2.8642x; 1.1763x over previous
"""DLRM forward (embedding gather + tiny MLPs) as a Bass/Tile kernel on 8 trn2 cores.

Sharding: data-parallel over the batch; each of the 8 cores handles B/8 = 2048
samples end-to-end against a full replica of the (read-only) tables.

Key transformation (host-side, exact): the top-MLP first layer is linear in the
embedding concat, so fold tw1 into the tables once per call:
    PT[f] = tables[f] @ tw1[f*64:(f+1)*64, :]        # [CARD, 16] per table
    hidden[b] = sum_f PT[f][idx[b,f]] + relu(x@w1+b1) @ (w2@tw1_d) + (b2@tw1_d + tb1)
The constant vector (b2@tw1_d + tb1) is folded into PT table 0. The device
then gathers 64B fp32 rows (26 per sample) and reduces them on DVE — no
[B, 26, 64] materialization, no PE transposes of embeddings, and the gather
descriptor stream (53248 64B rows/core) runs at the DMA floor.

Per-core pipeline:
  - 16 indirect DMAs (one per 128-sample tile) gather 26x16 f32 into one big
    SBUF tile; DVE reduces each [128, 26, 16] block over f (axis-X reduce on a
    strided view) into gathersum [128, 16].
  - bottom MLP runs feature-major on PE ([13,2048] loaded pre-transposed from
    host): w1 matmul -> relu -> (w2@tw1_d) matmul -> PE-transpose back to
    sample-major [128, 16] chunks in PSUM.
  - DVE adds gathersum + dense chunk, applies relu*tw2 in one
    scalar_tensor_tensor, reduces over the 16 hidden units; ACT applies
    sigmoid(+tb2); one final PE transpose lays y out [16, 128] for a single
    contiguous output DMA.
"""

import numpy as np

import concourse.bass as bass
import concourse.mybir as mybir
import concourse.tile as tile
from concourse import bacc

P = 128

# Problem constants (hardcoded per harness contract).
N_CORES = 8
B = 16384
F = 26
D = 64
DENSE = 13
CARD = 100000
H_BOT = 8
H_TOP = 16

f32 = mybir.dt.float32
i32 = mybir.dt.int32

GATHER_SIZES = [4, 4, 4, 2, 1, 1]  # tiles of 128 samples per indirect DMA


def build_kernel(
    b_loc=B // N_CORES,
    card=CARD,
    n_f=F,
    n_dense=DENSE,
    h_bot=H_BOT,
    h_top=H_TOP,
):
    v = n_f * card
    n_t = b_loc // P  # 16 tiles of 128 samples
    group = min(512, b_loc)  # batch columns per matmul group
    tpg = group // P  # tiles per group (4)
    n_g = b_loc // group  # groups (4)
    row = h_top  # gathered row length (16 f32)
    trow = n_f * row  # per-tile gather width (416)

    nc = bacc.Bacc("TRN2", target_bir_lowering=False)
    ptab_d = nc.dram_tensor("ptab", [v, row], f32, kind="ExternalInput")
    comb_d = nc.dram_tensor("comb", [P, n_t * n_f], i32, kind="ExternalInput")
    dxt_d = nc.dram_tensor("dxt", [n_dense, b_loc], f32, kind="ExternalInput")
    # wblob[:, 0:8] = w1 (13 rows); wblob[0:8, 8:24] = w2@tw1_d; wblob[0:8, 24] = b1
    wb_d = nc.dram_tensor("wblob", [n_dense, h_bot + h_top + 1], f32, kind="ExternalInput")
    # cblob[:, :128] = identity; cblob[:, 128:192] = tw2 tiled 4x; cblob[:, 192] = tb2
    cb_d = nc.dram_tensor("cblob", [P, P + tpg * h_top + 1], f32, kind="ExternalInput")
    y_d = nc.dram_tensor("y", [n_t, P], f32, kind="ExternalOutput")

    with tile.TileContext(nc) as tc:
        with (
            tc.tile_pool(name="const", bufs=1) as cpool,
            tc.tile_pool(name="small", bufs=2) as smallp,
            tc.tile_pool(name="pmm", bufs=2, space="PSUM") as pmmp,
            tc.tile_pool(name="pfix", bufs=1, space="PSUM") as pfixp,
        ):
            # ---- index upload on the gpsimd SWDGE queue itself, so the first
            # gather waits only on this DMA (not the whole sync-queue burst) ----
            comb = cpool.tile([P, n_t * n_f], i32)
            nc.gpsimd.dma_start(out=comb[:], in_=comb_d[:, :])

            # ---- constants via sync HWDGE queue (overlap the gather stream) ----
            cb = cpool.tile([P, P + tpg * h_top + 1], f32)
            nc.sync.dma_start(out=cb[:], in_=cb_d[:, :])
            ident = cb[:, 0:P]
            tw2b = cb[:, P : P + tpg * h_top]
            tb2b = cb[:, P + tpg * h_top : P + tpg * h_top + 1]
            wb = cpool.tile([n_dense, h_bot + h_top + 1], f32)
            nc.sync.dma_start(out=wb[:], in_=wb_d[:, :])
            w1_sb = wb[:, 0:h_bot]
            w21_sb = wb[0:h_bot, h_bot : h_bot + h_top]
            b1_sb = wb[0:h_bot, h_bot + h_top : h_bot + h_top + 1]
            dxt = cpool.tile([n_dense, b_loc], f32)
            nc.sync.dma_start(out=dxt[:], in_=dxt_d[:, :])

            big_et = cpool.tile([P, n_t * trow], f32)
            gs = cpool.tile([P, n_t * h_top], f32)  # gathersum [128, 256]
            hs = cpool.tile([P, n_t * h_top], f32)  # hidden pre-relu
            mm = cpool.tile([P, n_t * h_top], f32)  # relu(h) * tw2
            lg = cpool.tile([P, n_t], f32)  # logits [128, 16]
            ylog = cpool.tile([P, n_t], f32)
            pdhT = pfixp.tile([P, n_t * h_top], f32)  # dense hidden, sample-major

            # ---- bottom MLP, feature-major ----
            for g in range(n_g):
                ph = pmmp.tile([h_bot, group], f32, tag="pmm")
                nc.tensor.matmul(
                    out=ph[:],
                    lhsT=w1_sb,
                    rhs=dxt[:, bass.ts(g, group)],
                    start=True,
                    stop=True,
                )
                h1 = smallp.tile([h_bot, group], f32, tag="h1")
                nc.scalar.activation(
                    out=h1[:],
                    in_=ph[:],
                    func=mybir.ActivationFunctionType.Relu,
                    bias=b1_sb,
                )
                pd = pmmp.tile([h_top, group], f32, tag="pmm")
                nc.tensor.matmul(
                    out=pd[:], lhsT=w21_sb, rhs=h1[:], start=True, stop=True
                )
                dh = smallp.tile([h_top, group], f32, tag="dh")
                nc.scalar.activation(
                    out=dh[:],
                    in_=pd[:],
                    func=mybir.ActivationFunctionType.Copy,
                )
                for j in range(tpg):
                    nc.tensor.transpose(
                        out=pdhT[:, bass.ts(g * tpg + j, h_top)],
                        in_=dh[:, bass.ts(j, P)],
                        identity=ident[0:h_top, 0:h_top],
                    )

            # ---- gathers (ramped sizes: big first to amortize launch gaps,
            # small last to shorten the tail), reduces chasing each gather,
            # per-group head ops interleaved ----
            def emit_head(g):
                gcols = bass.ts(g, tpg * h_top)
                nc.vector.tensor_tensor(
                    out=hs[:, gcols],
                    in0=gs[:, gcols],
                    in1=pdhT[:, gcols],
                    op=mybir.AluOpType.add,
                )
                # relu then scale by tw2 (broadcast across partitions)
                nc.vector.scalar_tensor_tensor(
                    out=mm[:, gcols],
                    in0=hs[:, gcols],
                    scalar=0.0,
                    in1=tw2b,
                    op0=mybir.AluOpType.max,
                    op1=mybir.AluOpType.mult,
                )
                nc.vector.tensor_reduce(
                    out=lg[:, bass.ts(g, tpg)],
                    in_=mm[:, gcols].rearrange("p (t j) -> p t j", t=tpg),
                    axis=mybir.AxisListType.X,
                    op=mybir.AluOpType.add,
                )

            t0 = 0
            done_tiles = 0
            for sz in GATHER_SIZES:
                t1 = min(t0 + sz, n_t)
                nc.gpsimd.indirect_dma_start(
                    out=big_et[:, t0 * trow : t1 * trow],
                    out_offset=None,
                    in_=ptab_d[:, :],
                    in_offset=bass.IndirectOffsetOnAxis(
                        ap=comb[:, t0 * n_f : t1 * n_f], axis=0
                    ),
                )
                for t in range(t0, t1):
                    # sum over the 26 tables: [128, (f j)] -> [128, j]
                    nc.vector.tensor_reduce(
                        out=gs[:, bass.ts(t, h_top)],
                        in_=big_et[:, bass.ts(t, trow)].rearrange(
                            "p (f j) -> p j f", f=n_f
                        ),
                        axis=mybir.AxisListType.X,
                        op=mybir.AluOpType.add,
                    )
                    done_tiles += 1
                    if done_tiles % tpg == 0:
                        emit_head(done_tiles // tpg - 1)
                t0 = t1

            nc.scalar.activation(
                out=ylog[:],
                in_=lg[:],
                func=mybir.ActivationFunctionType.Sigmoid,
                bias=tb2b,
            )
            pyT = pfixp.tile([n_t, P], f32)
            nc.tensor.transpose(out=pyT[:], in_=ylog[:], identity=ident)
            yT = cpool.tile([n_t, P], f32)
            nc.vector.tensor_copy(out=yT[:], in_=pyT[:])
            nc.sync.dma_start(out=y_d[:, :], in_=yT[:])

    nc.compile()
    return nc


_NC_CACHE = {}


def _get_nc():
    if "nc" not in _NC_CACHE:
        _NC_CACHE["nc"] = build_kernel()
    return _NC_CACHE["nc"]


def make_in_maps(dense_x, sparse_x, tables, w1, b1, w2, b2, tw1, tb1, tw2, tb2):
    tables = np.asarray(tables, dtype=np.float32)
    tw1 = np.asarray(tw1, dtype=np.float32)
    tw2 = np.asarray(tw2, dtype=np.float32)
    w2 = np.asarray(w2, dtype=np.float32)
    b2 = np.asarray(b2, dtype=np.float32)
    tb1 = np.asarray(tb1, dtype=np.float32)
    tb2 = np.asarray(tb2, dtype=np.float32)

    # Fold tw1 into the tables: PT[f] = tables[f] @ tw1_f  -> [F, CARD, 16]
    tw1_e = tw1[: F * D].reshape(F, D, H_TOP)
    pt = np.einsum("fcd,fdh->fch", tables, tw1_e, optimize=True).astype(np.float32)
    # Fold the constant hidden-layer offset into table 0.
    c = (b2 @ tw1[F * D :]) + tb1  # [16]
    pt[0] += c
    ptab = np.ascontiguousarray(pt.reshape(F * CARD, H_TOP))

    w21 = np.ascontiguousarray(w2 @ tw1[F * D :])  # [8, 16]

    comb_full = np.asarray(sparse_x, dtype=np.int64) + (
        np.arange(F, dtype=np.int64) * CARD
    )
    comb_full = comb_full.astype(np.int32)  # [B, 26] flat PT row ids

    dense_f = np.asarray(dense_x, dtype=np.float32)

    tpg = 4
    cb = np.zeros((P, P + tpg * H_TOP + 1), dtype=np.float32)
    cb[:, :P] = np.eye(P, dtype=np.float32)
    cb[:, P : P + tpg * H_TOP] = np.tile(tw2.reshape(-1), tpg)
    cb[:, P + tpg * H_TOP] = tb2[0]

    wb = np.zeros((DENSE, H_BOT + H_TOP + 1), dtype=np.float32)
    wb[:, :H_BOT] = np.asarray(w1, np.float32)
    wb[:H_BOT, H_BOT : H_BOT + H_TOP] = w21
    wb[:H_BOT, H_BOT + H_TOP] = np.asarray(b1, np.float32)

    shared = {
        "ptab": ptab,
        "wblob": wb,
        "cblob": cb,
    }
    b_loc = B // N_CORES
    n_t = b_loc // P
    in_maps = []
    for c_ in range(N_CORES):
        lo, hi = c_ * b_loc, (c_ + 1) * b_loc
        comb_c = comb_full[lo:hi].reshape(n_t, P, F).transpose(1, 0, 2)
        m = dict(shared)
        m["comb"] = np.ascontiguousarray(comb_c.reshape(P, n_t * F))
        m["dxt"] = np.ascontiguousarray(dense_f[lo:hi].T)
        in_maps.append(m)
    return in_maps


def kernel(**inputs):
    from concourse.bass_utils import run_bass_kernel_spmd

    nc = _get_nc()
    in_maps = make_in_maps(**inputs)
    res = run_bass_kernel_spmd(nc, in_maps, core_ids=list(range(N_CORES)))
    out = np.concatenate([r["y"].reshape(-1) for r in res.results])
    return out.reshape(B, 1).astype(np.float32)


# revision 14
# speedup vs baseline: 3.0773x; 1.0744x over previous
"""DLRM forward (embedding gather + tiny MLPs) as a Bass/Tile kernel on 8 trn2 cores.

Sharding: data-parallel over the batch; each of the 8 cores handles B/8 = 2048
samples end-to-end against a full replica of the (read-only) tables.

Key transformation (host-side, exact): the top-MLP first layer is linear in the
embedding concat, so fold tw1 into the tables once per call:
    PT[f] = tables[f] @ tw1[f*64:(f+1)*64, :]        # [CARD, 16] per table
    hidden[b] = sum_f PT[f][idx[b,f]] + relu(x@w1+b1) @ (w2@tw1_d) + (b2@tw1_d + tb1)
The constant vector (b2@tw1_d + tb1) is folded into PT table 0. The device
then gathers 64B fp32 rows (26 per sample) and reduces them on DVE — no
[B, 26, 64] materialization, no PE transposes of embeddings, and the gather
descriptor stream (53248 64B rows/core) runs at the DMA floor.

Per-core pipeline:
  - 16 indirect DMAs (one per 128-sample tile) gather 26x16 f32 into one big
    SBUF tile; DVE reduces each [128, 26, 16] block over f (axis-X reduce on a
    strided view) into gathersum [128, 16].
  - bottom MLP runs feature-major on PE ([13,2048] loaded pre-transposed from
    host): w1 matmul -> relu -> (w2@tw1_d) matmul -> PE-transpose back to
    sample-major [128, 16] chunks in PSUM.
  - DVE adds gathersum + dense chunk, applies relu*tw2 in one
    scalar_tensor_tensor, reduces over the 16 hidden units; ACT applies
    sigmoid(+tb2); one final PE transpose lays y out [16, 128] for a single
    contiguous output DMA.
"""

import numpy as np

import concourse.bass as bass
import concourse.mybir as mybir
import concourse.tile as tile
from concourse import bacc

P = 128

# Problem constants (hardcoded per harness contract).
N_CORES = 8
B = 16384
F = 26
D = 64
DENSE = 13
CARD = 100000
H_BOT = 8
H_TOP = 16

f32 = mybir.dt.float32
i32 = mybir.dt.int32

GATHER_SIZES = [4, 4, 4, 4]  # tiles of 128 samples per indirect DMA
TABLE_DT = mybir.dt.float16
TABLE_NP_DT = np.float16


def build_kernel(
    b_loc=B // N_CORES,
    card=CARD,
    n_f=F,
    n_dense=DENSE,
    h_bot=H_BOT,
    h_top=H_TOP,
):
    v = n_f * card
    n_t = b_loc // P  # 16 tiles of 128 samples
    group = min(512, b_loc)  # batch columns per matmul group
    tpg = group // P  # tiles per group (4)
    n_g = b_loc // group  # groups (4)
    row = h_top  # gathered row length (16 f32)
    trow = n_f * row  # per-tile gather width (416)

    nc = bacc.Bacc("TRN2", target_bir_lowering=False)
    ptab_d = nc.dram_tensor("ptab", [v, row], TABLE_DT, kind="ExternalInput")
    comb_d = nc.dram_tensor("comb", [P, n_t * n_f], i32, kind="ExternalInput")
    dxt_d = nc.dram_tensor("dxt", [n_dense, b_loc], f32, kind="ExternalInput")
    # wblob[:, 0:8] = w1 (13 rows); wblob[0:8, 8:24] = w2@tw1_d; wblob[0:8, 24] = b1
    wb_d = nc.dram_tensor("wblob", [n_dense, h_bot + h_top + 1], f32, kind="ExternalInput")
    # cblob[:, :128] = identity; cblob[:, 128:192] = tw2 tiled 4x; cblob[:, 192] = tb2
    cb_d = nc.dram_tensor("cblob", [P, P + tpg * h_top + 1], f32, kind="ExternalInput")
    y_d = nc.dram_tensor("y", [n_t, P], f32, kind="ExternalOutput")

    with tile.TileContext(nc) as tc:
        with (
            tc.tile_pool(name="const", bufs=1) as cpool,
            tc.tile_pool(name="small", bufs=2) as smallp,
            tc.tile_pool(name="pmm", bufs=2, space="PSUM") as pmmp,
            tc.tile_pool(name="pd", bufs=2, space="PSUM") as pdp,
            tc.tile_pool(name="pfix", bufs=1, space="PSUM") as pfixp,
        ):
            # ---- index upload on the gpsimd SWDGE queue itself, so the first
            # gather waits only on this DMA (not the whole sync-queue burst);
            # split so the first gather's slice lands first ----
            c0 = GATHER_SIZES[0] * n_f
            comb = cpool.tile([P, n_t * n_f], i32)
            nc.gpsimd.dma_start(out=comb[:, 0:c0], in_=comb_d[:, 0:c0])

            # ---- constants via sync HWDGE queue (overlap the gather stream) ----
            dxt = cpool.tile([n_dense, b_loc], f32)
            nc.sync.dma_start(out=dxt[:], in_=dxt_d[:, :])
            wb = cpool.tile([n_dense, h_bot + h_top + 1], f32)
            nc.sync.dma_start(out=wb[:], in_=wb_d[:, :])
            w1_sb = wb[:, 0:h_bot]
            w21_sb = wb[0:h_bot, h_bot : h_bot + h_top]
            b1_sb = wb[0:h_bot, h_bot + h_top : h_bot + h_top + 1]
            cb = cpool.tile([P, P + tpg * h_top + 1], f32)
            nc.sync.dma_start(out=cb[:], in_=cb_d[:, :])
            ident = cb[:, 0:P]
            tw2b = cb[:, P : P + tpg * h_top]
            tb2b = cb[:, P + tpg * h_top : P + tpg * h_top + 1]

            big_et = cpool.tile([P, n_t * trow], TABLE_DT)
            gs = cpool.tile([P, n_t * h_top], f32)  # gathersum [128, 256]
            hs = cpool.tile([P, n_t * h_top], f32)  # hidden pre-relu
            mm = cpool.tile([P, n_t * h_top], f32)  # relu(h) * tw2
            lg = cpool.tile([P, n_t], f32)  # logits [128, 16]
            ylog = cpool.tile([P, n_t], f32)
            pdhT = pfixp.tile([P, n_t * h_top], f32)  # dense hidden, sample-major

            # ---- bottom MLP, feature-major, wave-scheduled so the in-order
            # PE queue never stalls on a not-yet-ready cross-engine dep ----
            phs, h1s, pds, dhs = [], [], [], []
            for g in range(n_g):
                ph = pmmp.tile([h_bot, group], f32, tag="pmm")
                nc.tensor.matmul(
                    out=ph[:],
                    lhsT=w1_sb,
                    rhs=dxt[:, bass.ts(g, group)],
                    start=True,
                    stop=True,
                )
                phs.append(ph)
            for g in range(n_g):
                h1 = smallp.tile([h_bot, group], f32, tag="h1")
                nc.scalar.activation(
                    out=h1[:],
                    in_=phs[g][:],
                    func=mybir.ActivationFunctionType.Relu,
                    bias=b1_sb,
                )
                h1s.append(h1)
            for g in range(n_g):
                pd = pdp.tile([h_top, group], f32, tag="pd")
                nc.tensor.matmul(
                    out=pd[:], lhsT=w21_sb, rhs=h1s[g][:], start=True, stop=True
                )
                pds.append(pd)
            for g in range(n_g):
                dh = smallp.tile([h_top, group], f32, tag="dh")
                nc.scalar.activation(
                    out=dh[:],
                    in_=pds[g][:],
                    func=mybir.ActivationFunctionType.Copy,
                )
                dhs.append(dh)
            for g in range(n_g):
                for j in range(tpg):
                    nc.tensor.transpose(
                        out=pdhT[:, bass.ts(g * tpg + j, h_top)],
                        in_=dhs[g][:, bass.ts(j, P)],
                        identity=ident[0:h_top, 0:h_top],
                    )

            # ---- gathers, one strided reduce chasing each gather,
            # per-group head ops interleaved ----
            def emit_head(g):
                gcols = bass.ts(g, tpg * h_top)
                nc.vector.tensor_tensor(
                    out=hs[:, gcols],
                    in0=gs[:, gcols],
                    in1=pdhT[:, gcols],
                    op=mybir.AluOpType.add,
                )
                # relu then scale by tw2 (broadcast across partitions)
                nc.vector.scalar_tensor_tensor(
                    out=mm[:, gcols],
                    in0=hs[:, gcols],
                    scalar=0.0,
                    in1=tw2b,
                    op0=mybir.AluOpType.max,
                    op1=mybir.AluOpType.mult,
                )
                nc.vector.tensor_reduce(
                    out=lg[:, bass.ts(g, tpg)],
                    in_=mm[:, gcols].rearrange("p (t j) -> p t j", t=tpg),
                    axis=mybir.AxisListType.X,
                    op=mybir.AluOpType.add,
                )

            t0 = 0
            done_tiles = 0
            for gi, sz in enumerate(GATHER_SIZES):
                t1 = min(t0 + sz, n_t)
                nc.gpsimd.indirect_dma_start(
                    out=big_et[:, t0 * trow : t1 * trow],
                    out_offset=None,
                    in_=ptab_d[:, :],
                    in_offset=bass.IndirectOffsetOnAxis(
                        ap=comb[:, t0 * n_f : t1 * n_f], axis=0
                    ),
                )
                if gi == 0:
                    # rest of the index upload, overlapped with gather 0's gen
                    nc.gpsimd.dma_start(
                        out=comb[:, c0 : n_t * n_f], in_=comb_d[:, c0 : n_t * n_f]
                    )
                # sum over the 26 tables: [128, (t f j)] -> [128, (t j)]
                nc.vector.tensor_reduce(
                    out=gs[:, t0 * h_top : t1 * h_top],
                    in_=big_et[:, t0 * trow : t1 * trow].rearrange(
                        "p (t f j) -> p t j f", t=t1 - t0, f=n_f
                    ),
                    axis=mybir.AxisListType.X,
                    op=mybir.AluOpType.add,
                )
                prev_groups = done_tiles // tpg
                done_tiles += t1 - t0
                for g in range(prev_groups, done_tiles // tpg):
                    emit_head(g)
                t0 = t1

            nc.scalar.activation(
                out=ylog[:],
                in_=lg[:],
                func=mybir.ActivationFunctionType.Sigmoid,
                bias=tb2b,
            )
            pyT = pfixp.tile([n_t, P], f32)
            nc.tensor.transpose(out=pyT[:], in_=ylog[:], identity=ident)
            yT = cpool.tile([n_t, P], f32)
            nc.vector.tensor_copy(out=yT[:], in_=pyT[:])
            nc.sync.dma_start(out=y_d[:, :], in_=yT[:])

    nc.compile()
    return nc


_NC_CACHE = {}


def _get_nc():
    if "nc" not in _NC_CACHE:
        _NC_CACHE["nc"] = build_kernel()
    return _NC_CACHE["nc"]


def make_in_maps(dense_x, sparse_x, tables, w1, b1, w2, b2, tw1, tb1, tw2, tb2):
    tables = np.asarray(tables, dtype=np.float32)
    tw1 = np.asarray(tw1, dtype=np.float32)
    tw2 = np.asarray(tw2, dtype=np.float32)
    w2 = np.asarray(w2, dtype=np.float32)
    b2 = np.asarray(b2, dtype=np.float32)
    tb1 = np.asarray(tb1, dtype=np.float32)
    tb2 = np.asarray(tb2, dtype=np.float32)

    # Fold tw1 into the tables: PT[f] = tables[f] @ tw1_f  -> [F, CARD, 16]
    tw1_e = tw1[: F * D].reshape(F, D, H_TOP)
    pt = np.einsum("fcd,fdh->fch", tables, tw1_e, optimize=True).astype(np.float32)
    # Fold the constant hidden-layer offset into table 0.
    c = (b2 @ tw1[F * D :]) + tb1  # [16]
    pt[0] += c
    ptab = np.ascontiguousarray(pt.reshape(F * CARD, H_TOP).astype(TABLE_NP_DT))

    w21 = np.ascontiguousarray(w2 @ tw1[F * D :])  # [8, 16]

    comb_full = np.asarray(sparse_x, dtype=np.int64) + (
        np.arange(F, dtype=np.int64) * CARD
    )
    comb_full = comb_full.astype(np.int32)  # [B, 26] flat PT row ids

    dense_f = np.asarray(dense_x, dtype=np.float32)

    tpg = 4
    cb = np.zeros((P, P + tpg * H_TOP + 1), dtype=np.float32)
    cb[:, :P] = np.eye(P, dtype=np.float32)
    cb[:, P : P + tpg * H_TOP] = np.tile(tw2.reshape(-1), tpg)
    cb[:, P + tpg * H_TOP] = tb2[0]

    wb = np.zeros((DENSE, H_BOT + H_TOP + 1), dtype=np.float32)
    wb[:, :H_BOT] = np.asarray(w1, np.float32)
    wb[:H_BOT, H_BOT : H_BOT + H_TOP] = w21
    wb[:H_BOT, H_BOT + H_TOP] = np.asarray(b1, np.float32)

    shared = {
        "ptab": ptab,
        "wblob": wb,
        "cblob": cb,
    }
    b_loc = B // N_CORES
    n_t = b_loc // P
    in_maps = []
    for c_ in range(N_CORES):
        lo, hi = c_ * b_loc, (c_ + 1) * b_loc
        comb_c = comb_full[lo:hi].reshape(n_t, P, F).transpose(1, 0, 2)
        m = dict(shared)
        m["comb"] = np.ascontiguousarray(comb_c.reshape(P, n_t * F))
        m["dxt"] = np.ascontiguousarray(dense_f[lo:hi].T)
        in_maps.append(m)
    return in_maps


def kernel(**inputs):
    from concourse.bass_utils import run_bass_kernel_spmd

    nc = _get_nc()
    in_maps = make_in_maps(**inputs)
    res = run_bass_kernel_spmd(nc, in_maps, core_ids=list(range(N_CORES)))
    out = np.concatenate([r["y"].reshape(-1) for r in res.results])
    return out.reshape(B, 1).astype(np.float32)


# revision 25
# speedup vs baseline: 3.1466x; 1.0225x over previous
"""DLRM forward (embedding gather + tiny MLPs) as a Bass/Tile kernel on 8 trn2 cores.

Sharding: data-parallel over the batch; each of the 8 cores handles B/8 = 2048
samples end-to-end against a full replica of the (read-only) tables.

Key transformation (host-side, exact): the top-MLP first layer is linear in the
embedding concat, so fold tw1 into the tables once per call:
    PT[f] = tables[f] @ tw1[f*64:(f+1)*64, :]        # [CARD, 16] per table
    hidden[b] = sum_f PT[f][idx[b,f]] + relu(x@w1+b1) @ (w2@tw1_d) + (b2@tw1_d + tb1)
The constant vector (b2@tw1_d + tb1) is folded into PT table 0. The device
then gathers 64B fp32 rows (26 per sample) and reduces them on DVE — no
[B, 26, 64] materialization, no PE transposes of embeddings, and the gather
descriptor stream (53248 64B rows/core) runs at the DMA floor.

Per-core pipeline:
  - 16 indirect DMAs (one per 128-sample tile) gather 26x16 f32 into one big
    SBUF tile; DVE reduces each [128, 26, 16] block over f (axis-X reduce on a
    strided view) into gathersum [128, 16].
  - bottom MLP runs feature-major on PE ([13,2048] loaded pre-transposed from
    host): w1 matmul -> relu -> (w2@tw1_d) matmul -> PE-transpose back to
    sample-major [128, 16] chunks in PSUM.
  - DVE adds gathersum + dense chunk, applies relu*tw2 in one
    scalar_tensor_tensor, reduces over the 16 hidden units; ACT applies
    sigmoid(+tb2); one final PE transpose lays y out [16, 128] for a single
    contiguous output DMA.
"""

import numpy as np

import concourse.bass as bass
import concourse.mybir as mybir
import concourse.tile as tile
from concourse import bacc

P = 128

# Problem constants (hardcoded per harness contract).
N_CORES = 8
B = 16384
F = 26
D = 64
DENSE = 13
CARD = 100000
H_BOT = 8
H_TOP = 16

f32 = mybir.dt.float32
i32 = mybir.dt.int32

GATHER_SIZES = [1, 1, 2, 4, 4, 4]  # tiles of 128 samples per indirect DMA
TABLE_DT = mybir.dt.float16
TABLE_NP_DT = np.float16
f16 = mybir.dt.float16


def build_kernel(
    b_loc=B // N_CORES,
    card=CARD,
    n_f=F,
    n_dense=DENSE,
    h_bot=H_BOT,
    h_top=H_TOP,
):
    v = n_f * card
    n_t = b_loc // P  # 16 tiles of 128 samples
    group = min(512, b_loc)  # batch columns per matmul group
    tpg = group // P  # tiles per group (4)
    n_g = b_loc // group  # groups (4)
    row = h_top  # gathered row length (16 f32)
    trow = n_f * row  # per-tile gather width (416)

    nc = bacc.Bacc("TRN2", target_bir_lowering=False)
    ptab_d = nc.dram_tensor("ptab", [v, row], TABLE_DT, kind="ExternalInput")
    comb_d = nc.dram_tensor("comb", [P, n_t * n_f], i32, kind="ExternalInput")
    dxt_d = nc.dram_tensor("dxt", [n_dense, b_loc], f16, kind="ExternalInput")
    # wblob[:, 0:8] = w1 (13 rows); wblob[0:8, 8:24] = w2@tw1_d  (fp16)
    wb_d = nc.dram_tensor("wblob", [n_dense, h_bot + h_top], f16, kind="ExternalInput")
    # cblob[:, 0:64] = tw2 tiled 4x; cblob[:, 64] = tb2; cblob[0:8, 65] = b1
    cb_d = nc.dram_tensor("cblob", [P, tpg * h_top + 2], f32, kind="ExternalInput")
    id_d = nc.dram_tensor("identh", [P, P], f16, kind="ExternalInput")
    y_d = nc.dram_tensor("y", [n_t, P], f32, kind="ExternalOutput")

    with tile.TileContext(nc) as tc:
        with (
            tc.tile_pool(name="const", bufs=1) as cpool,
            tc.tile_pool(name="small", bufs=2) as smallp,
            tc.tile_pool(name="pmm", bufs=2, space="PSUM") as pmmp,
            tc.tile_pool(name="pd", bufs=2, space="PSUM") as pdp,
            tc.tile_pool(name="pfix", bufs=1, space="PSUM") as pfixp,
        ):
            # ---- index upload: sole occupant of the sync (SP) queue, so the
            # first gather waits on exactly one DMA; split so the first
            # gather's slice lands first ----
            c0 = GATHER_SIZES[0] * n_f
            comb = cpool.tile([P, n_t * n_f], i32)
            nc.sync.dma_start(out=comb[:, 0:c0], in_=comb_d[:, 0:c0])
            nc.sync.dma_start(out=comb[:, c0 : n_t * n_f], in_=comb_d[:, c0 : n_t * n_f])

            # ---- constants via the Activation HWDGE queue (independent) ----
            dxt = cpool.tile([n_dense, b_loc], f16)
            nc.scalar.dma_start(out=dxt[:], in_=dxt_d[:, :])
            wb = cpool.tile([n_dense, h_bot + h_top], f16)
            nc.scalar.dma_start(out=wb[:], in_=wb_d[:, :])
            w1_sb = wb[:, 0:h_bot]
            w21_sb = wb[0:h_bot, h_bot : h_bot + h_top]
            cb = cpool.tile([P, tpg * h_top + 2], f32)
            nc.scalar.dma_start(out=cb[:], in_=cb_d[:, :])
            tw2b = cb[:, 0 : tpg * h_top]
            tb2b = cb[:, tpg * h_top : tpg * h_top + 1]
            b1_sb = cb[0:h_bot, tpg * h_top + 1 : tpg * h_top + 2]
            ident = cpool.tile([P, P], f16)
            nc.scalar.dma_start(out=ident[:], in_=id_d[:, :])

            big_et = cpool.tile([P, n_t * trow], TABLE_DT)
            gs = cpool.tile([P, n_t * h_top], f32)  # gathersum [128, 256]
            hs = cpool.tile([P, n_t * h_top], f32)  # hidden pre-relu
            mm = cpool.tile([P, n_t * h_top], f32)  # relu(h) * tw2
            lg = cpool.tile([P, n_t], f32)  # logits [128, 16]
            ylog = cpool.tile([P, n_t], f16)
            pdhT = pfixp.tile([P, n_t * h_top], f16)  # dense hidden, sample-major

            # ---- bottom MLP, feature-major, wave-scheduled so the in-order
            # PE queue never stalls on a not-yet-ready cross-engine dep ----
            phs, h1s, pds, dhs = [], [], [], []
            for g in range(n_g):
                ph = pmmp.tile([h_bot, group], f32, tag="pmm")
                nc.tensor.matmul(
                    out=ph[:],
                    lhsT=w1_sb,
                    rhs=dxt[:, bass.ts(g, group)],
                    start=True,
                    stop=True,
                )
                phs.append(ph)
            for g in range(n_g):
                h1 = smallp.tile([h_bot, group], f16, tag="h1")
                nc.scalar.activation(
                    out=h1[:],
                    in_=phs[g][:],
                    func=mybir.ActivationFunctionType.Relu,
                    bias=b1_sb,
                )
                h1s.append(h1)
            for g in range(n_g):
                pd = pdp.tile([h_top, group], f32, tag="pd")
                nc.tensor.matmul(
                    out=pd[:], lhsT=w21_sb, rhs=h1s[g][:], start=True, stop=True
                )
                pds.append(pd)
            for g in range(n_g):
                dh = smallp.tile([h_top, group], f16, tag="dh")
                nc.scalar.activation(
                    out=dh[:],
                    in_=pds[g][:],
                    func=mybir.ActivationFunctionType.Copy,
                )
                dhs.append(dh)
            for g in range(n_g):
                for j in range(tpg):
                    nc.tensor.transpose(
                        out=pdhT[:, bass.ts(g * tpg + j, h_top)],
                        in_=dhs[g][:, bass.ts(j, P)],
                        identity=ident[0:h_top, 0:h_top],
                    )

            # ---- gathers, one strided reduce chasing each gather,
            # per-group head ops interleaved ----
            def emit_head(g):
                gcols = bass.ts(g, tpg * h_top)
                nc.vector.tensor_tensor(
                    out=hs[:, gcols],
                    in0=gs[:, gcols],
                    in1=pdhT[:, gcols],
                    op=mybir.AluOpType.add,
                )
                # relu then scale by tw2 (broadcast across partitions)
                nc.vector.scalar_tensor_tensor(
                    out=mm[:, gcols],
                    in0=hs[:, gcols],
                    scalar=0.0,
                    in1=tw2b,
                    op0=mybir.AluOpType.max,
                    op1=mybir.AluOpType.mult,
                )
                nc.vector.tensor_reduce(
                    out=lg[:, bass.ts(g, tpg)],
                    in_=mm[:, gcols].rearrange("p (t j) -> p t j", t=tpg),
                    axis=mybir.AxisListType.X,
                    op=mybir.AluOpType.add,
                )

            t0 = 0
            done_tiles = 0
            for gi, sz in enumerate(GATHER_SIZES):
                t1 = min(t0 + sz, n_t)
                nc.gpsimd.indirect_dma_start(
                    out=big_et[:, t0 * trow : t1 * trow],
                    out_offset=None,
                    in_=ptab_d[:, :],
                    in_offset=bass.IndirectOffsetOnAxis(
                        ap=comb[:, t0 * n_f : t1 * n_f], axis=0
                    ),
                )
                # sum over the 26 tables, one DVE reduce per 128-sample tile
                for t in range(t0, t1):
                    nc.vector.tensor_reduce(
                        out=gs[:, bass.ts(t, h_top)],
                        in_=big_et[:, bass.ts(t, trow)].rearrange(
                            "p (f j) -> p j f", f=n_f
                        ),
                        axis=mybir.AxisListType.X,
                        op=mybir.AluOpType.add,
                    )
                prev_groups = done_tiles // tpg
                done_tiles += t1 - t0
                for g in range(prev_groups, done_tiles // tpg):
                    emit_head(g)
                t0 = t1

            nc.scalar.activation(
                out=ylog[:],
                in_=lg[:],
                func=mybir.ActivationFunctionType.Sigmoid,
                bias=tb2b,
            )
            pyT = pfixp.tile([n_t, P], f16)
            nc.tensor.transpose(out=pyT[:], in_=ylog[:], identity=ident[:])
            yT = cpool.tile([n_t, P], f32)
            nc.vector.tensor_copy(out=yT[:], in_=pyT[:])
            nc.sync.dma_start(out=y_d[:, :], in_=yT[:])

    nc.compile()
    return nc


_NC_CACHE = {}


def _get_nc():
    if "nc" not in _NC_CACHE:
        _NC_CACHE["nc"] = build_kernel()
    return _NC_CACHE["nc"]


def make_in_maps(dense_x, sparse_x, tables, w1, b1, w2, b2, tw1, tb1, tw2, tb2):
    tables = np.asarray(tables, dtype=np.float32)
    tw1 = np.asarray(tw1, dtype=np.float32)
    tw2 = np.asarray(tw2, dtype=np.float32)
    w2 = np.asarray(w2, dtype=np.float32)
    b2 = np.asarray(b2, dtype=np.float32)
    tb1 = np.asarray(tb1, dtype=np.float32)
    tb2 = np.asarray(tb2, dtype=np.float32)

    # Fold tw1 into the tables: PT[f] = tables[f] @ tw1_f  -> [F, CARD, 16]
    tw1_e = tw1[: F * D].reshape(F, D, H_TOP)
    pt = np.einsum("fcd,fdh->fch", tables, tw1_e, optimize=True).astype(np.float32)
    # Fold the constant hidden-layer offset into table 0.
    c = (b2 @ tw1[F * D :]) + tb1  # [16]
    pt[0] += c
    ptab = np.ascontiguousarray(pt.reshape(F * CARD, H_TOP).astype(TABLE_NP_DT))

    w21 = np.ascontiguousarray(w2 @ tw1[F * D :])  # [8, 16]

    comb_full = np.asarray(sparse_x, dtype=np.int64) + (
        np.arange(F, dtype=np.int64) * CARD
    )
    comb_full = comb_full.astype(np.int32)  # [B, 26] flat PT row ids

    dense_f = np.asarray(dense_x, dtype=np.float32)

    tpg = 4
    cb = np.zeros((P, tpg * H_TOP + 2), dtype=np.float32)
    cb[:, : tpg * H_TOP] = np.tile(tw2.reshape(-1), tpg)
    cb[:, tpg * H_TOP] = tb2[0]
    cb[:H_BOT, tpg * H_TOP + 1] = np.asarray(b1, np.float32)

    wb = np.zeros((DENSE, H_BOT + H_TOP), dtype=np.float16)
    wb[:, :H_BOT] = np.asarray(w1, np.float32).astype(np.float16)
    wb[:H_BOT, H_BOT : H_BOT + H_TOP] = w21.astype(np.float16)

    shared = {
        "ptab": ptab,
        "wblob": wb,
        "cblob": cb,
        "identh": np.eye(P, dtype=np.float16),
    }
    b_loc = B // N_CORES
    n_t = b_loc // P
    in_maps = []
    for c_ in range(N_CORES):
        lo, hi = c_ * b_loc, (c_ + 1) * b_loc
        comb_c = comb_full[lo:hi].reshape(n_t, P, F).transpose(1, 0, 2)
        m = dict(shared)
        m["comb"] = np.ascontiguousarray(comb_c.reshape(P, n_t * F))
        m["dxt"] = np.ascontiguousarray(dense_f[lo:hi].T.astype(np.float16))
        in_maps.append(m)
    return in_maps


def kernel(**inputs):
    from concourse.bass_utils import run_bass_kernel_spmd

    nc = _get_nc()
    in_maps = make_in_maps(**inputs)
    res = run_bass_kernel_spmd(nc, in_maps, core_ids=list(range(N_CORES)))
    out = np.concatenate([r["y"].reshape(-1) for r in res.results])
    return out.reshape(B, 1).astype(np.float32)


# revision 31
# speedup vs baseline: 3.4010x; 1.0809x over previous
"""DLRM forward (embedding gather + tiny MLPs) as a Bass/Tile kernel on 8 trn2 cores.

Sharding: data-parallel over the batch; each of the 8 cores handles B/8 = 2048
samples end-to-end against a full replica of the (read-only) tables.

Key transformation (host-side, exact): the top-MLP first layer is linear in the
embedding concat, so fold tw1 into the tables once per call:
    PT[f] = tables[f] @ tw1[f*64:(f+1)*64, :]        # [CARD, 16] per table
    hidden[b] = sum_f PT[f][idx[b,f]] + relu(x@w1+b1) @ (w2@tw1_d) + (b2@tw1_d + tb1)
The constant vector (b2@tw1_d + tb1) is folded into PT table 0. The device
then gathers 64B fp32 rows (26 per sample) and reduces them on DVE — no
[B, 26, 64] materialization, no PE transposes of embeddings, and the gather
descriptor stream (53248 64B rows/core) runs at the DMA floor.

Per-core pipeline:
  - 16 indirect DMAs (one per 128-sample tile) gather 26x16 f32 into one big
    SBUF tile; DVE reduces each [128, 26, 16] block over f (axis-X reduce on a
    strided view) into gathersum [128, 16].
  - bottom MLP runs feature-major on PE ([13,2048] loaded pre-transposed from
    host): w1 matmul -> relu -> (w2@tw1_d) matmul -> PE-transpose back to
    sample-major [128, 16] chunks in PSUM.
  - DVE adds gathersum + dense chunk, applies relu*tw2 in one
    scalar_tensor_tensor, reduces over the 16 hidden units; ACT applies
    sigmoid(+tb2); one final PE transpose lays y out [16, 128] for a single
    contiguous output DMA.
"""

import numpy as np

import concourse.bass as bass
import concourse.mybir as mybir
import concourse.tile as tile
from concourse import bacc

P = 128

# Problem constants (hardcoded per harness contract).
N_CORES = 8
B = 16384
F = 26
D = 64
DENSE = 13
CARD = 100000
H_BOT = 8
H_TOP = 16

f32 = mybir.dt.float32
i32 = mybir.dt.int32

GATHER_SIZES = [1, 1, 2, 4, 4, 4]  # tiles of 128 samples per indirect DMA
POOL_TILES = 4  # trailing 128-sample tiles whose 26-sum runs on gpsimd
TABLE_DT = mybir.dt.float16
TABLE_NP_DT = np.float16
f16 = mybir.dt.float16


def build_kernel(
    b_loc=B // N_CORES,
    card=CARD,
    n_f=F,
    n_dense=DENSE,
    h_bot=H_BOT,
    h_top=H_TOP,
):
    v = n_f * card
    n_t = b_loc // P  # 16 tiles of 128 samples
    group = min(512, b_loc)  # batch columns per matmul group
    tpg = group // P  # tiles per group (4)
    n_g = b_loc // group  # groups (4)
    row = h_top  # gathered row length (16 f32)
    trow = n_f * row  # per-tile gather width (416)

    nc = bacc.Bacc("TRN2", target_bir_lowering=False)
    ptab_d = nc.dram_tensor("ptab", [v, row], TABLE_DT, kind="ExternalInput")
    comb_d = nc.dram_tensor("comb", [P, n_t * n_f], i32, kind="ExternalInput")
    dxt_d = nc.dram_tensor("dxt", [n_dense, b_loc], f16, kind="ExternalInput")
    # wblob[:, 0:8] = w1 (13 rows); wblob[0:8, 8:24] = w2@tw1_d  (fp16)
    wb_d = nc.dram_tensor("wblob", [n_dense, h_bot + h_top], f16, kind="ExternalInput")
    # cblob[:, 0:64] = tw2 tiled 4x; cblob[:, 64] = tb2; cblob[0:8, 65] = b1
    cb_d = nc.dram_tensor("cblob", [P, tpg * h_top + 2], f32, kind="ExternalInput")
    id_d = nc.dram_tensor("identh", [P, P], f16, kind="ExternalInput")
    y_d = nc.dram_tensor("y", [n_t, P], f32, kind="ExternalOutput")

    with tile.TileContext(nc) as tc:
        with (
            tc.tile_pool(name="const", bufs=1) as cpool,
            tc.tile_pool(name="small", bufs=2) as smallp,
            tc.tile_pool(name="pmm", bufs=2, space="PSUM") as pmmp,
            tc.tile_pool(name="pd", bufs=2, space="PSUM") as pdp,
            tc.tile_pool(name="pfix", bufs=1, space="PSUM") as pfixp,
        ):
            # ---- index upload: sole occupant of the sync (SP) queue, so the
            # first gather waits on exactly one DMA; split so the first
            # gather's slice lands first ----
            c0 = GATHER_SIZES[0] * n_f
            comb = cpool.tile([P, n_t * n_f], i32)
            nc.sync.dma_start(out=comb[:, 0:c0], in_=comb_d[:, 0:c0])
            nc.sync.dma_start(out=comb[:, c0 : n_t * n_f], in_=comb_d[:, c0 : n_t * n_f])

            # ---- constants via the Activation HWDGE queue (independent) ----
            dxt = cpool.tile([n_dense, b_loc], f16)
            nc.scalar.dma_start(out=dxt[:], in_=dxt_d[:, :])
            wb = cpool.tile([n_dense, h_bot + h_top], f16)
            nc.scalar.dma_start(out=wb[:], in_=wb_d[:, :])
            w1_sb = wb[:, 0:h_bot]
            w21_sb = wb[0:h_bot, h_bot : h_bot + h_top]
            cb = cpool.tile([P, tpg * h_top + 2], f32)
            nc.scalar.dma_start(out=cb[:], in_=cb_d[:, :])
            tw2b = cb[:, 0 : tpg * h_top]
            tb2b = cb[:, tpg * h_top : tpg * h_top + 1]
            b1_sb = cb[0:h_bot, tpg * h_top + 1 : tpg * h_top + 2]
            ident = cpool.tile([P, P], f16)
            nc.scalar.dma_start(out=ident[:], in_=id_d[:, :])

            big_et = cpool.tile([P, n_t * trow], TABLE_DT)
            gs = cpool.tile([P, n_t * h_top], f32)  # gathersum [128, 256]
            hs = cpool.tile([P, n_t * h_top], f32)  # hidden pre-relu
            mm = cpool.tile([P, n_t * h_top], f32)  # relu(h) * tw2
            lg = cpool.tile([P, n_t], f32)  # logits [128, 16]
            ylog = cpool.tile([P, n_t], f16)
            pdhT = pfixp.tile([P, n_t * h_top], f16)  # dense hidden, sample-major

            # ---- bottom MLP, feature-major, wave-scheduled so the in-order
            # PE queue never stalls on a not-yet-ready cross-engine dep ----
            phs, h1s, pds, dhs = [], [], [], []
            for g in range(n_g):
                ph = pmmp.tile([h_bot, group], f32, tag="pmm")
                nc.tensor.matmul(
                    out=ph[:],
                    lhsT=w1_sb,
                    rhs=dxt[:, bass.ts(g, group)],
                    start=True,
                    stop=True,
                )
                phs.append(ph)
            for g in range(n_g):
                h1 = smallp.tile([h_bot, group], f16, tag="h1")
                nc.scalar.activation(
                    out=h1[:],
                    in_=phs[g][:],
                    func=mybir.ActivationFunctionType.Relu,
                    bias=b1_sb,
                )
                h1s.append(h1)
            for g in range(n_g):
                pd = pdp.tile([h_top, group], f32, tag="pd")
                nc.tensor.matmul(
                    out=pd[:], lhsT=w21_sb, rhs=h1s[g][:], start=True, stop=True
                )
                pds.append(pd)
            for g in range(n_g):
                dh = smallp.tile([h_top, group], f16, tag="dh")
                nc.scalar.activation(
                    out=dh[:],
                    in_=pds[g][:],
                    func=mybir.ActivationFunctionType.Copy,
                )
                dhs.append(dh)
            for g in range(n_g):
                for j in range(tpg):
                    nc.tensor.transpose(
                        out=pdhT[:, bass.ts(g * tpg + j, h_top)],
                        in_=dhs[g][:, bass.ts(j, P)],
                        identity=ident[0:h_top, 0:h_top],
                    )

            # ---- gathers, one strided reduce chasing each gather,
            # per-group head ops interleaved ----
            def emit_head(g):
                gcols = bass.ts(g, tpg * h_top)
                nc.vector.tensor_tensor(
                    out=hs[:, gcols],
                    in0=gs[:, gcols],
                    in1=pdhT[:, gcols],
                    op=mybir.AluOpType.add,
                )
                # relu then scale by tw2 (broadcast across partitions)
                nc.vector.scalar_tensor_tensor(
                    out=mm[:, gcols],
                    in0=hs[:, gcols],
                    scalar=0.0,
                    in1=tw2b,
                    op0=mybir.AluOpType.max,
                    op1=mybir.AluOpType.mult,
                )
                nc.vector.tensor_reduce(
                    out=lg[:, bass.ts(g, tpg)],
                    in_=mm[:, gcols].rearrange("p (t j) -> p t j", t=tpg),
                    axis=mybir.AxisListType.X,
                    op=mybir.AluOpType.add,
                )

            n_dve = n_t - POOL_TILES
            t0 = 0
            done_tiles = 0
            for gi, sz in enumerate(GATHER_SIZES):
                t1 = min(t0 + sz, n_t)
                nc.gpsimd.indirect_dma_start(
                    out=big_et[:, t0 * trow : t1 * trow],
                    out_offset=None,
                    in_=ptab_d[:, :],
                    in_offset=bass.IndirectOffsetOnAxis(
                        ap=comb[:, t0 * n_f : t1 * n_f], axis=0
                    ),
                )
                # sum over the 26 tables, one DVE reduce per 128-sample tile
                for t in range(t0, min(t1, n_dve)):
                    nc.vector.tensor_reduce(
                        out=gs[:, bass.ts(t, h_top)],
                        in_=big_et[:, bass.ts(t, trow)].rearrange(
                            "p (f j) -> p j f", f=n_f
                        ),
                        axis=mybir.AxisListType.X,
                        op=mybir.AluOpType.add,
                    )
                prev_groups = done_tiles // tpg
                done_tiles += t1 - t0
                for g in range(prev_groups, min(done_tiles, n_dve) // tpg):
                    emit_head(g)
                t0 = t1

            # ---- trailing tiles: 26-sum + head on gpsimd (idle after desc
            # gen), in parallel with DVE's tiles ----
            if POOL_TILES:
                K = POOL_TILES
                base = n_dve * trow
                pool_et = big_et[:, base : n_t * trow]
                va = pool_et.rearrange("p (t f j) -> p t f j", t=K, f=n_f)
                scrA = cpool.tile([P, K * 13 * h_top], f16)
                vA = scrA[:].rearrange("p (t f j) -> p t f j", t=K, f=13)
                nc.gpsimd.tensor_tensor(
                    out=vA, in0=va[:, :, 0:13], in1=va[:, :, 13:26],
                    op=mybir.AluOpType.add,
                )
                scrB = cpool.tile([P, K * 6 * h_top], f16)
                vB = scrB[:].rearrange("p (t f j) -> p t f j", t=K, f=6)
                nc.gpsimd.tensor_tensor(
                    out=vB, in0=vA[:, :, 0:6], in1=vA[:, :, 6:12],
                    op=mybir.AluOpType.add,
                )
                scrC = cpool.tile([P, K * 3 * h_top], f16)
                vC = scrC[:].rearrange("p (t f j) -> p t f j", t=K, f=3)
                nc.gpsimd.tensor_tensor(
                    out=vC, in0=vB[:, :, 0:3], in1=vB[:, :, 3:6],
                    op=mybir.AluOpType.add,
                )
                scrD = cpool.tile([P, K * h_top], f16)
                vD = scrD[:].rearrange("p (t c j) -> p t c j", t=K, c=1)
                nc.gpsimd.tensor_tensor(
                    out=vD, in0=vC[:, :, 0:1], in1=vC[:, :, 1:2],
                    op=mybir.AluOpType.add,
                )
                nc.gpsimd.tensor_tensor(
                    out=vD, in0=vD, in1=vC[:, :, 2:3], op=mybir.AluOpType.add
                )
                gsl = gs[:, n_dve * h_top : n_t * h_top].rearrange(
                    "p (t c j) -> p t c j", t=K, c=1
                )
                nc.gpsimd.tensor_tensor(
                    out=gsl, in0=vD, in1=vA[:, :, 12:13], op=mybir.AluOpType.add
                )
                # head for the gpsimd-owned tiles (assumes K == tpg);
                # gpsimd cannot read PSUM, so stage dense-hidden via ACT
                dhT_sb = cpool.tile([P, K * h_top], f32)
                nc.scalar.activation(
                    out=dhT_sb[:],
                    in_=pdhT[:, n_dve * h_top : n_t * h_top],
                    func=mybir.ActivationFunctionType.Copy,
                )
                hsl = hs[:, n_dve * h_top : n_t * h_top]
                nc.gpsimd.tensor_tensor(
                    out=hsl,
                    in0=gs[:, n_dve * h_top : n_t * h_top],
                    in1=dhT_sb[:],
                    op=mybir.AluOpType.add,
                )
                # relu*tw2 + reduce for these tiles runs on DVE (appended
                # after its own queue; TensorScalarPtr is not a Pool opcode)
                g_last = n_dve // tpg
                gcols = bass.ts(g_last, tpg * h_top)
                nc.vector.scalar_tensor_tensor(
                    out=mm[:, gcols],
                    in0=hs[:, gcols],
                    scalar=0.0,
                    in1=tw2b,
                    op0=mybir.AluOpType.max,
                    op1=mybir.AluOpType.mult,
                )
                nc.vector.tensor_reduce(
                    out=lg[:, bass.ts(g_last, tpg)],
                    in_=mm[:, gcols].rearrange("p (t j) -> p t j", t=tpg),
                    axis=mybir.AxisListType.X,
                    op=mybir.AluOpType.add,
                )

            nc.scalar.activation(
                out=ylog[:],
                in_=lg[:],
                func=mybir.ActivationFunctionType.Sigmoid,
                bias=tb2b,
            )
            pyT = pfixp.tile([n_t, P], f16)
            nc.tensor.transpose(out=pyT[:], in_=ylog[:], identity=ident[:])
            yT = cpool.tile([n_t, P], f32)
            nc.vector.tensor_copy(out=yT[:], in_=pyT[:])
            nc.sync.dma_start(out=y_d[:, :], in_=yT[:])

    nc.compile()
    return nc


_NC_CACHE = {}


def _get_nc():
    if "nc" not in _NC_CACHE:
        _NC_CACHE["nc"] = build_kernel()
    return _NC_CACHE["nc"]


def make_in_maps(dense_x, sparse_x, tables, w1, b1, w2, b2, tw1, tb1, tw2, tb2):
    tables = np.asarray(tables, dtype=np.float32)
    tw1 = np.asarray(tw1, dtype=np.float32)
    tw2 = np.asarray(tw2, dtype=np.float32)
    w2 = np.asarray(w2, dtype=np.float32)
    b2 = np.asarray(b2, dtype=np.float32)
    tb1 = np.asarray(tb1, dtype=np.float32)
    tb2 = np.asarray(tb2, dtype=np.float32)

    # Fold tw1 into the tables: PT[f] = tables[f] @ tw1_f  -> [F, CARD, 16]
    tw1_e = tw1[: F * D].reshape(F, D, H_TOP)
    pt = np.einsum("fcd,fdh->fch", tables, tw1_e, optimize=True).astype(np.float32)
    # Fold the constant hidden-layer offset into table 0.
    c = (b2 @ tw1[F * D :]) + tb1  # [16]
    pt[0] += c
    ptab = np.ascontiguousarray(pt.reshape(F * CARD, H_TOP).astype(TABLE_NP_DT))

    w21 = np.ascontiguousarray(w2 @ tw1[F * D :])  # [8, 16]

    comb_full = np.asarray(sparse_x, dtype=np.int64) + (
        np.arange(F, dtype=np.int64) * CARD
    )
    comb_full = comb_full.astype(np.int32)  # [B, 26] flat PT row ids

    dense_f = np.asarray(dense_x, dtype=np.float32)

    tpg = 4
    cb = np.zeros((P, tpg * H_TOP + 2), dtype=np.float32)
    cb[:, : tpg * H_TOP] = np.tile(tw2.reshape(-1), tpg)
    cb[:, tpg * H_TOP] = tb2[0]
    cb[:H_BOT, tpg * H_TOP + 1] = np.asarray(b1, np.float32)

    wb = np.zeros((DENSE, H_BOT + H_TOP), dtype=np.float16)
    wb[:, :H_BOT] = np.asarray(w1, np.float32).astype(np.float16)
    wb[:H_BOT, H_BOT : H_BOT + H_TOP] = w21.astype(np.float16)

    shared = {
        "ptab": ptab,
        "wblob": wb,
        "cblob": cb,
        "identh": np.eye(P, dtype=np.float16),
    }
    b_loc = B // N_CORES
    n_t = b_loc // P
    in_maps = []
    for c_ in range(N_CORES):
        lo, hi = c_ * b_loc, (c_ + 1) * b_loc
        comb_c = comb_full[lo:hi].reshape(n_t, P, F).transpose(1, 0, 2)
        m = dict(shared)
        m["comb"] = np.ascontiguousarray(comb_c.reshape(P, n_t * F))
        m["dxt"] = np.ascontiguousarray(dense_f[lo:hi].T.astype(np.float16))
        in_maps.append(m)
    return in_maps


def kernel(**inputs):
    from concourse.bass_utils import run_bass_kernel_spmd

    nc = _get_nc()
    in_maps = make_in_maps(**inputs)
    res = run_bass_kernel_spmd(nc, in_maps, core_ids=list(range(N_CORES)))
    out = np.concatenate([r["y"].reshape(-1) for r in res.results])
    return out.reshape(B, 1).astype(np.float32)
